# revision 32
# baseline (speedup 1.0000x reference)
"""Two-layer GAT encoder on 8 Trainium2 NeuronCores (Bass/Tile).

Strategy (graph/data parallel, dst-partitioned), v3:
  - Nodes are partitioned contiguously across the 8 cores (6250 each); every
    edge lives on the core that owns its destination node, so softmax
    segments never cross cores.
  - Layer 1 uses NO device-side gathers: the host ships xT pre-gathered per
    edge (src and dst column slabs, window-ordered).  Per 128-edge block the
    PE computes [h+b | alpha_src] = gxs_block^T @ W1ext and alpha_dst =
    gxd_block^T @ vd1 into one PSUM tile.  dma_gather costs ~5-9ns of serial
    GpSimd time PER INDEX (Q7 software descriptor generation), so replacing
    the layer-1 feature+alpha gathers (~230k idxs/core) with linear slab
    streams + matmuls removes ~1.6ms of serial GpSimd work.
  - Layer 2 gathers h2 rows of edge sources from the AllGather'd table2
    (256B rows, src is device-computed so host pre-gather is impossible),
    but the per-edge alpha_dst lookup is computed on-chip:
    alpha_d[e] = reduce_d(onehot[e,d] * adrep[e,d]) via one fused
    tensor_tensor_reduce per block (single head), with adrep built per
    group by a PE outer-product broadcast of the window's ad column.
  - Per 128-dst-node window: one-hot segment matrix via iota/is_equal (bf16),
    exp-weighted features + softmax denominator accumulated in PSUM by a
    chain of bf16 matmuls, deferred division.
  - Softmax runs without max-subtraction: |alpha_s + alpha_d| < ~8 for this
    architecture (weights scaled by 0.1), so exp() is safe in fp32.
"""

from contextlib import ExitStack

import numpy as np
import ml_dtypes

import concourse.bass as bass
import concourse.bacc as bacc
import concourse.tile as tile
import concourse.mybir as mybir
from concourse.bass_utils import run_bass_kernel_spmd
from concourse.masks import make_identity

F32 = mybir.dt.float32
BF16 = mybir.dt.bfloat16
I16 = mybir.dt.int16

N = 50000
NCORES = 8
NPC = N // NCORES          # 6250 nodes per core
WIN = 128
NWIN = (NPC + WIN - 1) // WIN   # 49
LASTW = NPC - (NWIN - 1) * WIN  # 106 valid rows in the final window
G = 3                      # windows per group
F1, H1 = 128, 4
F2, H2 = 32, 1
T2C = 128                  # table2 row cols (bf16): [h2+b2(32) | as2 | ad2 | pad]
SPLIT = 32768              # int16 index ceiling for gather views
SG = 3                     # layer-1 blocks per PSUM bank (3*136 <= 512 f32)

GROUPS = [tuple(range(g, min(g + G, NWIN))) for g in range(0, NWIN, G)]
BF = ml_dtypes.bfloat16


def _wrap_idx(a):
    """dma_gather idx layout: index i -> partition i%16, col i//16,
    replicated across the 8 groups of 16 partitions."""
    n = len(a)
    assert n % 16 == 0
    w = a.reshape(n // 16, 16).T
    return np.tile(w, (8, 1)).astype(np.int16)


def _build_structures(edge_index, x):
    """Partition/sort edges; emit per-core slabs and shared meta."""
    src = np.concatenate([edge_index[0], np.arange(N, dtype=np.int64)]).astype(np.int32)
    dst = np.concatenate([edge_index[1], np.arange(N, dtype=np.int64)]).astype(np.int32)
    xT = np.ascontiguousarray(np.asarray(x, np.float32).T.astype(BF))  # [128, N]

    cores = []
    for c in range(NCORES):
        m = (dst >= c * NPC) & (dst < (c + 1) * NPC)
        s = src[m]
        dl = dst[m] - c * NPC
        w = dl >> 7
        order = np.argsort(w, kind='stable')
        s, dl, w = s[order], dl[order], w[order]
        cnt = np.bincount(w, minlength=NWIN)
        off = np.concatenate([[0], np.cumsum(cnt)])
        cores.append((s, dl, cnt, off))

    # ---- layer 1: per-window padded blocks, no sections -------------------
    Bw = [max(-(-int(cores[c][2][w]) // 128) for c in range(NCORES))
          for w in range(NWIN)]
    l1_groups = []
    col = 0
    for wins in GROUPS:
        nb = sum(Bw[w] for w in wins)
        l1_groups.append(dict(base=wins[0] * WIN, wins=tuple(wins),
                              bw=tuple(Bw[w] for w in wins), nb=nb, col=col))
        col += nb
    l1_cols = col

    slabs = [dict() for _ in range(NCORES)]
    for c in range(NCORES):
        s, dl, cnt, off = cores[c]
        gxs = np.zeros((128, l1_cols * 128), dtype=BF)
        gxd = np.zeros((128, l1_cols * 128), dtype=BF)
        rel = np.full((128, l1_cols), 200.0, dtype=BF)
        for gm in l1_groups:
            pos = gm['col'] * 128
            for wi, w in enumerate(gm['wins']):
                k = int(cnt[w])
                sl = slice(off[w], off[w] + k)
                gxs[:, pos:pos + k] = xT[:, s[sl]]
                gxd[:, pos:pos + k] = xT[:, dl[sl] + c * NPC]
                r = np.full(gm['bw'][wi] * 128, 200.0, dtype=np.float32)
                r[:k] = dl[sl] - w * WIN
                rel[:, pos // 128:pos // 128 + gm['bw'][wi]] = \
                    np.ascontiguousarray(
                        r.reshape(gm['bw'][wi], 128).T).astype(BF)
                pos += gm['bw'][wi] * 128
        slabs[c]['gxs'] = gxs
        slabs[c]['gxd'] = gxd
        slabs[c]['rel1'] = rel

    # ---- layer 2: lo/hi sections for int16 gather idxs --------------------
    l2_groups = []
    sorted_cores = []
    for c in range(NCORES):
        s, dl, cnt, off = cores[c]
        sec = (s >= SPLIT).astype(np.int32)
        w = dl >> 7
        order = np.lexsort((sec, w))
        ks, dls = s[order], dl[order]
        cnt2 = np.bincount(w[order] * 2 + sec[order], minlength=NWIN * 2)
        off2 = np.concatenate([[0], np.cumsum(cnt2)])
        sorted_cores.append((ks, dls, cnt2, off2))

    icol = 0
    rcol = 0
    for wins in GROUPS:
        base = wins[0] * WIN
        Blo = [max(-(-int(sorted_cores[c][2][w * 2]) // 128)
                   for c in range(NCORES)) for w in wins]
        Bhi = [max(-(-int(sorted_cores[c][2][w * 2 + 1]) // 128)
                   for c in range(NCORES)) for w in wins]
        nlo, nhi = 128 * sum(Blo), 128 * sum(Bhi)
        ntot = nlo + nhi
        wcols = []
        lo_off, hi_off = 0, nlo // 128
        for wi, w in enumerate(wins):
            cols = (list(range(lo_off, lo_off + Blo[wi]))
                    + list(range(hi_off, hi_off + Bhi[wi])))
            wcols.append(tuple(cols))
            lo_off += Blo[wi]
            hi_off += Bhi[wi]
        l2_groups.append(dict(base=base, nlo=nlo, nhi=nhi, ntot=ntot,
                              wins=tuple(wins), wcols=tuple(wcols),
                              blo=tuple(Blo), bhi=tuple(Bhi),
                              icol=icol, rcol=rcol))
        icol += ntot // 16
        rcol += ntot // 128
    l2_icols, l2_rcols = icol, rcol

    for c in range(NCORES):
        ks, dls, cnt2, off2 = sorted_cores[c]
        idx_parts = []
        rel_parts = []
        for gm in l2_groups:
            ntot = gm['ntot']
            gi = np.zeros(ntot, dtype=np.int32)
            rel = np.full(ntot, 200.0, dtype=np.float32)
            pos = 0
            for pass_hi in (0, 1):
                for wi, w in enumerate(gm['wins']):
                    k = int(cnt2[w * 2 + pass_hi])
                    sl = slice(off2[w * 2 + pass_hi],
                               off2[w * 2 + pass_hi] + k)
                    gi[pos:pos + k] = ks[sl] - (SPLIT if pass_hi else 0)
                    rel[pos:pos + k] = dls[sl] - w * WIN
                    pos += (gm['bhi'] if pass_hi else gm['blo'])[wi] * 128
            idx_parts.append(_wrap_idx(gi))
            rel_parts.append(np.ascontiguousarray(
                rel.reshape(ntot // 128, 128).T).astype(BF))
        slabs[c]['idx2'] = np.ascontiguousarray(
            np.concatenate(idx_parts, axis=1))
        slabs[c]['rel2'] = np.ascontiguousarray(
            np.concatenate(rel_parts, axis=1))

    meta = dict(l1_groups=l1_groups, l1_cols=l1_cols,
                l2_groups=l2_groups, l2_icols=l2_icols, l2_rcols=l2_rcols)
    key = (tuple((g['base'], g['wins'], g['bw'], g['nb'])
                 for g in l1_groups),
           tuple((g['base'], g['nlo'], g['nhi'], g['wins'], g['wcols'])
                 for g in l2_groups))
    return slabs, meta, key


def gat_program(ctx, tc, ins, outs, meta):
    if isinstance(meta, tuple):
        meta = meta[0]
    nc = tc.nc

    t2loc = nc.dram_tensor("t2loc", [NPC, T2C], BF16)
    adT = nc.dram_tensor("adT", [1, NPC], BF16)
    table2 = nc.dram_tensor("table2", [N, T2C], BF16, addr_space="Shared")

    const = ctx.enter_context(tc.tile_pool(name="const", bufs=1))
    gp = ctx.enter_context(tc.tile_pool(name="gp", bufs=2))
    sm = ctx.enter_context(tc.tile_pool(name="sm", bufs=2))
    ohp = ctx.enter_context(tc.tile_pool(name="ohp", bufs=2))
    wp = ctx.enter_context(tc.tile_pool(name="wp", bufs=2))
    pe = ctx.enter_context(tc.tile_pool(name="pe", bufs=4, space="PSUM"))
    pw = ctx.enter_context(tc.tile_pool(name="pw", bufs=2, space="PSUM"))
    pt = ctx.enter_context(tc.tile_pool(name="pt", bufs=1, space="PSUM"))
    pp = ctx.enter_context(tc.tile_pool(name="pp", bufs=1))

    # --- resident constants -------------------------------------------------
    w1t = const.tile([128, F1 + H1], BF16)
    nc.sync.dma_start(out=w1t[:], in_=ins['w1ext'])
    vd1t = const.tile([128, H1], BF16)
    nc.sync.dma_start(out=vd1t[:], in_=ins['vd1'])
    w2t = const.tile([128, F2 + 2 * H2], BF16)
    nc.sync.dma_start(out=w2t[:], in_=ins['w2ext'])
    b1t = const.tile([128, F1], F32)
    nc.sync.dma_start(out=b1t[:], in_=ins['bias1B'])
    b2t = const.tile([128, F2 + 2 * H2], F32)
    nc.sync.dma_start(out=b2t[:], in_=ins['bias2B'])
    onest = const.tile([1, 128], BF16)
    nc.vector.memset(onest[:], 1.0)
    ident = const.tile([128, 128], BF16)
    make_identity(nc, ident[:])
    iota_i = const.tile([128, 128], mybir.dt.int32)
    nc.gpsimd.iota(iota_i[:], pattern=[[1, 128]], base=0, channel_multiplier=0)
    iotaf = const.tile([128, 128], BF16)
    nc.vector.tensor_copy(out=iotaf[:], in_=iota_i[:])

    # --- layer-1 sink: ELU -> transpose -> table2 row chunk ----------------
    def l1_sink(w, nrows, res):
        mn = sm.tile([128, F1], F32, tag="mn")
        nc.vector.tensor_scalar_min(mn[:], res[:], 0.0)
        en = sm.tile([128, F1], F32, tag="en")
        nc.scalar.activation(out=en[:], in_=mn[:],
                             func=mybir.ActivationFunctionType.Exp)
        mp = sm.tile([128, F1], F32, tag="mp")
        nc.vector.tensor_scalar(
            out=mp[:], in0=res[:], scalar1=0.0, scalar2=-1.0,
            op0=mybir.AluOpType.max, op1=mybir.AluOpType.add)
        hp = sm.tile([128, F1], BF16, tag="hp")
        nc.vector.tensor_tensor(
            out=hp[:], in0=en[:], in1=mp[:], op=mybir.AluOpType.add)
        pst = pt.tile([128, 128], BF16, tag="pst")
        nc.tensor.transpose(out=pst[:], in_=hp[:], identity=ident[:])
        hpt = sm.tile([128, 128], BF16, tag="hpt")
        nc.vector.tensor_copy(out=hpt[:], in_=pst[:])
        ps2 = pt.tile([128, F2 + 2 * H2], F32, tag="ps2")
        nc.tensor.matmul(out=ps2[:], lhsT=hpt[:], rhs=w2t[:],
                         start=True, stop=True)
        t2 = sm.tile([128, F2 + 2 * H2], BF16, tag="t2")
        nc.vector.tensor_tensor(
            out=t2[:], in0=ps2[:], in1=b2t[:, 0:F2 + 2 * H2],
            op=mybir.AluOpType.add)
        nc.sync.dma_start(
            out=t2loc.ap()[w * 128:w * 128 + nrows, 0:F2 + 2 * H2],
            in_=t2[:nrows, :])
        # transposed ad2 column for layer-2 adrep broadcasts
        nc.sync.dma_start(
            out=adT.ap()[0:1, w * 128:w * 128 + nrows]
                .rearrange("x n -> n x"),
            in_=t2[:nrows, F2 + H2:F2 + 2 * H2])

    # =======================================================================
    # Layer 1: host-pregathered x slabs -> PE expansion, no gathers
    # =======================================================================
    def l1_consume(gm, oh, wm):
        bcol = 0
        for wi, w in enumerate(gm['wins']):
            nrows = LASTW if w == NWIN - 1 else 128
            bw = gm['bw'][wi]
            pswt = pw.tile([128, 256], F32, tag="psw")
            psw = pswt[:, 0:F1 + H1]
            for j in range(bw):
                nc.tensor.matmul(
                    out=psw, lhsT=oh[:, bcol + j, :], rhs=wm[:, bcol + j, :],
                    start=(j == 0), stop=(j == bw - 1),
                )
            bcol += bw
            rec = sm.tile([128, H1], F32, tag="rec")
            nc.vector.reciprocal(rec[:nrows], psw[:nrows, F1:F1 + H1])
            res = sm.tile([128, F1], F32, tag="res")
            if nrows < 128:
                nc.vector.memset(res[:], 0.0)
            nc.vector.tensor_tensor(
                out=res[:nrows, :].rearrange("p (h c) -> p h c", h=H1),
                in0=psw[:nrows, 0:F1].rearrange("p (h c) -> p h c", h=H1),
                in1=rec[:nrows].unsqueeze(2)
                    .broadcast_to([nrows, H1, F1 // H1]),
                op=mybir.AluOpType.mult,
            )
            nc.vector.tensor_tensor(
                out=res[:nrows, :], in0=res[:nrows, :], in1=b1t[:nrows, :],
                op=mybir.AluOpType.add)
            l1_sink(w, nrows, res)

    pending = None
    for l1gi, gm in enumerate(meta['l1_groups']):
        nb, col = gm['nb'], gm['col']
        gxs = gp.tile([128, nb * 128], BF16, tag="gxs")
        nc.sync.dma_start(out=gxs[:], in_=ins['gxs'][:, col * 128:(col + nb) * 128])
        gxd = gp.tile([128, nb * 128], BF16, tag="gxd")
        nc.sync.dma_start(out=gxd[:], in_=ins['gxd'][:, col * 128:(col + nb) * 128])
        relt = sm.tile([128, nb], BF16, tag="relt")
        nc.sync.dma_start(out=relt[:], in_=ins['rel1'][:, col:col + nb])

        oh = ohp.tile([128, nb, 128], BF16, tag="oh")
        nc.vector.tensor_tensor(
            out=oh[:],
            in0=relt[:].unsqueeze(2).broadcast_to([128, nb, 128]),
            in1=iotaf[:].unsqueeze(1).broadcast_to([128, nb, 128]),
            op=mybir.AluOpType.is_equal,
        )

        wm = wp.tile([128, nb, F1 + H1], BF16, tag="wm")
        # process in PSUM sub-batches of SG blocks
        for s0 in range(0, nb, SG):
            sn = min(SG, nb - s0)
            psA = pe.tile([128, SG, F1 + H1], F32, tag="psA")
            for j in range(sn):
                b = s0 + j
                nc.tensor.matmul(
                    out=psA[:, j, 0:F1 + H1],
                    lhsT=gxs[:, b * 128:(b + 1) * 128],
                    rhs=w1t[:], start=True, stop=False)
                # accumulate alpha_dst onto the alpha_src columns: the
                # PSUM region [F1:F1+H1] ends up holding s = a_src + a_dst
                nc.tensor.matmul(
                    out=psA[:, j, F1:F1 + H1],
                    lhsT=gxd[:, b * 128:(b + 1) * 128],
                    rhs=vd1t[:], start=False, stop=True)
            # leaky_relu(s) = max(0.2*s, s); each op reads PSUM only once
            t1 = sm.tile([128, SG, H1], F32, tag="t1")
            nc.vector.tensor_scalar_mul(t1[:, :sn, :],
                                        psA[:, :sn, F1:F1 + H1], 0.2)
            e = sm.tile([128, SG, H1], F32, tag="e")
            nc.vector.tensor_tensor(
                out=e[:, :sn, :], in0=psA[:, :sn, F1:F1 + H1],
                in1=t1[:, :sn, :], op=mybir.AluOpType.max)
            exb = sm.tile([128, SG, H1], BF16, tag="exb")
            nc.scalar.activation(out=exb[:, :sn, :], in_=e[:, :sn, :],
                                 func=mybir.ActivationFunctionType.Exp)
            nc.vector.tensor_tensor(
                out=wm[:, s0:s0 + sn, 0:F1]
                    .rearrange("p b (h c) -> p b h c", h=H1),
                in0=psA[:, :sn, 0:F1]
                    .rearrange("p b (h c) -> p b h c", h=H1),
                in1=exb[:, :sn, :].unsqueeze(3)
                    .broadcast_to([128, sn, H1, F1 // H1]),
                op=mybir.AluOpType.mult,
            )
            nc.vector.tensor_copy(out=wm[:, s0:s0 + sn, F1:F1 + H1],
                                  in_=exb[:, :sn, :])

        # software pipeline: aggregate+sink the PREVIOUS group so its
        # aggregation matmuls overlap this group's expansion/vector work
        if pending is not None:
            l1_consume(*pending)
        pending = (gm, oh, wm)
    l1_consume(*pending)

    nc.gpsimd.collective_compute(
        "AllGather", mybir.AluOpType.bypass,
        replica_groups=[list(range(NCORES))],
        ins=[t2loc.ap().opt()], outs=[table2.ap().opt()],
    )

    # =======================================================================
    # Layer 2: gather table2 rows by src; alpha_dst via onehot-reduce
    # =======================================================================
    qrr = [0]
    MAXG = 3840
    FD, HD = F2, H2
    for gm in meta['l2_groups']:
        nlo, nhi, ntot, base = gm['nlo'], gm['nhi'], gm['ntot'], gm['base']
        nb = ntot // 128
        icol, rcol = gm['icol'], gm['rcol']
        idxt = sm.tile([128, ntot // 16], I16, tag="idxt")
        nc.sync.dma_start(out=idxt[:], in_=ins['idx2'][:, icol:icol + ntot // 16])
        relt = sm.tile([128, nb], BF16, tag="relt2")
        nc.sync.dma_start(out=relt[:], in_=ins['rel2'][:, rcol:rcol + nb])

        def chunked_gather(out_tile, col0, in_ap, icol0, n, ec):
            for off in range(0, n, MAXG):
                k = min(MAXG, n - off)
                c = col0 + off // 128
                nc.gpsimd.dma_gather(
                    out_ap=out_tile[:, c:c + k // 128, :],
                    in_ap=in_ap,
                    idxs_ap=idxt[:, icol0 + off // 16:
                                 icol0 + (off + k) // 16],
                    num_idxs=k, num_idxs_reg=k,
                    elem_size=ec, elem_step=ec,
                    single_packet=False,
                    queue_num=qrr[0],
                )
                qrr[0] = (qrr[0] + 1) % 4

        g = gp.tile([128, nb, T2C], BF16, tag="g2")
        if nlo:
            chunked_gather(g, 0, table2.ap()[0:SPLIT, :], 0, nlo, T2C)
        if nhi:
            chunked_gather(g, nlo // 128, table2.ap()[SPLIT:N, :],
                           nlo // 16, nhi, T2C)

        # adrep: [128, G*WIN] replicated ad2 row via PE outer product
        adend = min(base + G * WIN, NPC)
        adw = adend - base
        adv = sm.tile([1, G * WIN], BF16, tag="adv")
        nc.sync.dma_start(out=adv[:, :adw], in_=adT.ap()[0:1, base:adend])
        adrep = sm.tile([128, G * WIN], BF16, tag="adrep")
        if adw < G * WIN:
            nc.vector.memset(adrep[:], 0.0)
        nc.gpsimd.partition_broadcast(adrep[:, :adw], adv[:, :adw])

        oh = ohp.tile([128, nb, 128], BF16, tag="oh2")
        nc.vector.tensor_tensor(
            out=oh[:],
            in0=relt[:].unsqueeze(2).broadcast_to([128, nb, 128]),
            in1=iotaf[:].unsqueeze(1).broadcast_to([128, nb, 128]),
            op=mybir.AluOpType.is_equal,
        )
        # alpha_d per edge: reduce_d(oh * adrep_window), batched over the
        # contiguous lo/hi block runs of each window
        ad_e = sm.tile([128, nb, 1], F32, tag="ad_e")
        prod = pp.tile([128, nb, 128], BF16, tag="prod")
        runs = []
        lo_acc, hi_acc = 0, nlo // 128
        for wi, w in enumerate(gm['wins']):
            woff = (w - gm['wins'][0]) * 128
            if gm['blo'][wi]:
                runs.append((lo_acc, gm['blo'][wi], woff))
            if gm['bhi'][wi]:
                runs.append((hi_acc, gm['bhi'][wi], woff))
            lo_acc += gm['blo'][wi]
            hi_acc += gm['bhi'][wi]
        for b0, r, woff in runs:
            nc.vector.tensor_tensor(
                out=prod[:, b0:b0 + r, :], in0=oh[:, b0:b0 + r, :],
                in1=adrep[:, woff:woff + 128].unsqueeze(1)
                    .broadcast_to([128, r, 128]),
                op=mybir.AluOpType.mult)
            nc.vector.tensor_reduce(
                out=ad_e[:, b0:b0 + r, :], in_=prod[:, b0:b0 + r, :],
                axis=mybir.AxisListType.X, op=mybir.AluOpType.add)
        s = sm.tile([128, nb, HD], F32, tag="s2")
        nc.vector.tensor_tensor(
            out=s[:], in0=g[:, :, FD:FD + H2],
            in1=ad_e[:], op=mybir.AluOpType.add)
        e = sm.tile([128, nb, HD], F32, tag="e2")
        nc.vector.scalar_tensor_tensor(
            out=e[:], in0=s[:], scalar=0.2, in1=s[:],
            op0=mybir.AluOpType.mult, op1=mybir.AluOpType.max)
        exb = sm.tile([128, nb, HD], BF16, tag="exb2")
        nc.scalar.activation(out=exb[:], in_=e[:],
                             func=mybir.ActivationFunctionType.Exp)
        wm = wp.tile([128, nb, FD + HD], BF16, tag="wm2")
        nc.vector.tensor_tensor(
            out=wm[:, :, 0:FD],
            in0=g[:, :, 0:FD],
            in1=exb[:].broadcast_to([128, nb, FD]),
            op=mybir.AluOpType.mult,
        )
        nc.vector.tensor_copy(out=wm[:, :, FD:FD + HD], in_=exb[:])

        for wi, w in enumerate(gm['wins']):
            nrows = LASTW if w == NWIN - 1 else 128
            cols = gm['wcols'][wi]
            pswt = pw.tile([128, 256], F32, tag="psw")
            psw = pswt[:, 0:FD + HD]
            for j, b in enumerate(cols):
                nc.tensor.matmul(
                    out=psw, lhsT=oh[:, b, :], rhs=wm[:, b, :],
                    start=(j == 0), stop=(j == len(cols) - 1),
                )
            rec = sm.tile([128, HD], F32, tag="rec2")
            nc.vector.reciprocal(rec[:nrows], psw[:nrows, FD:FD + HD])
            res = sm.tile([128, FD], F32, tag="res2")
            nc.vector.tensor_tensor(
                out=res[:nrows, :],
                in0=psw[:nrows, 0:FD],
                in1=rec[:nrows].broadcast_to([nrows, FD]),
                op=mybir.AluOpType.mult,
            )
            nc.sync.dma_start(
                out=outs['out'][w * 128:w * 128 + nrows, :],
                in_=res[:nrows, 0:FD])


def prepare_host_inputs(x, edge_index, W1, a_src1, a_dst1, b1, W2, a_src2,
                        a_dst2, b2):
    slabs, meta, key = _build_structures(np.asarray(edge_index), x)
    v_s1 = np.einsum('ihc,hc->ih', np.asarray(W1).reshape(F1, H1, F1 // H1),
                     np.asarray(a_src1)).astype(np.float32)
    v_d1 = np.einsum('ihc,hc->ih', np.asarray(W1).reshape(F1, H1, F1 // H1),
                     np.asarray(a_dst1)).astype(np.float32)
    v_s2 = np.einsum('ihc,hc->ih', np.asarray(W2).reshape(F1, H2, F2 // H2),
                     np.asarray(a_src2)).astype(np.float32)
    v_d2 = np.einsum('ihc,hc->ih', np.asarray(W2).reshape(F1, H2, F2 // H2),
                     np.asarray(a_dst2)).astype(np.float32)
    w1ext = np.concatenate(
        [np.asarray(W1, np.float32), v_s1], axis=1).astype(BF)
    w2ext = np.concatenate(
        [np.asarray(W2, np.float32), v_s2, v_d2], axis=1).astype(BF)
    bias1B = np.ascontiguousarray(np.broadcast_to(
        np.asarray(b1, np.float32), (128, F1)))
    bias2 = np.concatenate([np.asarray(b2, np.float32),
                            np.zeros(2 * H2, np.float32)])
    bias2B = np.ascontiguousarray(np.broadcast_to(bias2, (128, F2 + 2 * H2)))
    shared = dict(w1ext=w1ext, vd1=v_d1.astype(BF), w2ext=w2ext,
                  bias1B=bias1B, bias2B=bias2B)
    in_maps = []
    for c in range(NCORES):
        in_maps.append(dict(
            shared,
            gxs=slabs[c]['gxs'], gxd=slabs[c]['gxd'], rel1=slabs[c]['rel1'],
            idx2=slabs[c]['idx2'], rel2=slabs[c]['rel2'],
        ))
    return in_maps, (meta, key)


_PROGRAM_CACHE = {}


def _build_full_program(meta_key):
    meta, key = meta_key
    if key in _PROGRAM_CACHE:
        return _PROGRAM_CACHE[key]
    nc = bacc.Bacc(trn_type="TRN2", num_devices=NCORES, debug=False,
                   num_swdge_queues=4)
    ins = {
        'gxs': nc.dram_tensor('gxs', [128, meta['l1_cols'] * 128], BF16,
                              kind="ExternalInput").ap(),
        'gxd': nc.dram_tensor('gxd', [128, meta['l1_cols'] * 128], BF16,
                              kind="ExternalInput").ap(),
        'rel1': nc.dram_tensor('rel1', [128, meta['l1_cols']], BF16,
                               kind="ExternalInput").ap(),
        'idx2': nc.dram_tensor('idx2', [128, meta['l2_icols']], I16,
                               kind="ExternalInput").ap(),
        'rel2': nc.dram_tensor('rel2', [128, meta['l2_rcols']], BF16,
                               kind="ExternalInput").ap(),
        'w1ext': nc.dram_tensor('w1ext', [128, F1 + H1], BF16,
                                kind="ExternalInput").ap(),
        'vd1': nc.dram_tensor('vd1', [128, H1], BF16,
                              kind="ExternalInput").ap(),
        'w2ext': nc.dram_tensor('w2ext', [128, F2 + 2 * H2], BF16,
                                kind="ExternalInput").ap(),
        'bias1B': nc.dram_tensor('bias1B', [128, F1], F32,
                                 kind="ExternalInput").ap(),
        'bias2B': nc.dram_tensor('bias2B', [128, F2 + 2 * H2], F32,
                                 kind="ExternalInput").ap(),
    }
    outs = {'out': nc.dram_tensor('out', [NPC, F2], F32,
                                  kind="ExternalOutput").ap()}
    with tile.TileContext(nc) as tc:
        with ExitStack() as ctx:
            gat_program(ctx, tc, ins, outs, meta)
    nc.compile()
    _PROGRAM_CACHE[key] = nc
    return nc


def kernel(**inputs) -> np.ndarray:
    in_maps, meta_key = prepare_host_inputs(**inputs)
    nc = _build_full_program(meta_key)
    res = run_bass_kernel_spmd(nc, in_maps, core_ids=list(range(NCORES)))
    return np.concatenate([r['out'] for r in res.results], axis=0)


# revision 37
# speedup vs baseline: 1.2397x; 1.2397x over previous
"""Two-layer GAT encoder on 8 Trainium2 NeuronCores (Bass/Tile).

Strategy (graph/data parallel, dst-partitioned), v3:
  - Nodes are partitioned contiguously across the 8 cores (6250 each); every
    edge lives on the core that owns its destination node, so softmax
    segments never cross cores.
  - Layer 1 uses NO device-side gathers: the host ships xT pre-gathered per
    edge (src and dst column slabs, window-ordered).  Per 128-edge block the
    PE computes [h+b | alpha_src] = gxs_block^T @ W1ext and alpha_dst =
    gxd_block^T @ vd1 into one PSUM tile.  dma_gather costs ~5-9ns of serial
    GpSimd time PER INDEX (Q7 software descriptor generation), so replacing
    the layer-1 feature+alpha gathers (~230k idxs/core) with linear slab
    streams + matmuls removes ~1.6ms of serial GpSimd work.
  - Layer 2 gathers h2 rows of edge sources from the AllGather'd table2
    (256B rows, src is device-computed so host pre-gather is impossible),
    but the per-edge alpha_dst lookup is computed on-chip:
    alpha_d[e] = reduce_d(onehot[e,d] * adrep[e,d]) via one fused
    tensor_tensor_reduce per block (single head), with adrep built per
    group by a PE outer-product broadcast of the window's ad column.
  - Per 128-dst-node window: one-hot segment matrix via iota/is_equal (bf16),
    exp-weighted features + softmax denominator accumulated in PSUM by a
    chain of bf16 matmuls, deferred division.
  - Softmax runs without max-subtraction: |alpha_s + alpha_d| < ~8 for this
    architecture (weights scaled by 0.1), so exp() is safe in fp32.
"""

from contextlib import ExitStack

import numpy as np
import ml_dtypes

import concourse.bass as bass
import concourse.bacc as bacc
import concourse.tile as tile
import concourse.mybir as mybir
from concourse.bass_utils import run_bass_kernel_spmd
from concourse.masks import make_identity

F32 = mybir.dt.float32
BF16 = mybir.dt.bfloat16
I16 = mybir.dt.int16

N = 50000
NCORES = 8
NPC = N // NCORES          # 6250 nodes per core
WIN = 128
NWIN = (NPC + WIN - 1) // WIN   # 49
LASTW = NPC - (NWIN - 1) * WIN  # 106 valid rows in the final window
G = 2                      # windows per group
F1, H1 = 128, 4
F2, H2 = 32, 1
T2C = 128                  # table2 row cols (bf16): [h2+b2(32) | as2 | ad2 | pad]
SPLIT = 32768              # int16 index ceiling for gather views
SG = 3                     # layer-1 blocks per PSUM bank (3*136 <= 512 f32)

GROUPS = [tuple(range(g, min(g + G, NWIN))) for g in range(0, NWIN, G)]
BF = ml_dtypes.bfloat16


def _wrap_idx(a):
    """dma_gather idx layout: index i -> partition i%16, col i//16,
    replicated across the 8 groups of 16 partitions."""
    n = len(a)
    assert n % 16 == 0
    w = a.reshape(n // 16, 16).T
    return np.tile(w, (8, 1)).astype(np.int16)


def _build_structures(edge_index, x):
    """Partition/sort edges; emit per-core slabs and shared meta."""
    src = np.concatenate([edge_index[0], np.arange(N, dtype=np.int64)]).astype(np.int32)
    dst = np.concatenate([edge_index[1], np.arange(N, dtype=np.int64)]).astype(np.int32)
    xT = np.ascontiguousarray(np.asarray(x, np.float32).T.astype(BF))  # [128, N]

    cores = []
    for c in range(NCORES):
        m = (dst >= c * NPC) & (dst < (c + 1) * NPC)
        s = src[m]
        dl = dst[m] - c * NPC
        w = dl >> 7
        order = np.argsort(w, kind='stable')
        s, dl, w = s[order], dl[order], w[order]
        cnt = np.bincount(w, minlength=NWIN)
        off = np.concatenate([[0], np.cumsum(cnt)])
        cores.append((s, dl, cnt, off))

    # ---- layer 1: per-window padded blocks, no sections -------------------
    Bw = [max(-(-int(cores[c][2][w]) // 128) for c in range(NCORES))
          for w in range(NWIN)]
    l1_groups = []
    col = 0
    for wins in GROUPS:
        nb = sum(Bw[w] for w in wins)
        l1_groups.append(dict(base=wins[0] * WIN, wins=tuple(wins),
                              bw=tuple(Bw[w] for w in wins), nb=nb, col=col))
        col += nb
    l1_cols = col

    slabs = [dict() for _ in range(NCORES)]
    for c in range(NCORES):
        s, dl, cnt, off = cores[c]
        gxs = np.zeros((128, l1_cols * 128), dtype=BF)
        gxd = np.zeros((128, l1_cols * 128), dtype=BF)
        rel = np.full((128, l1_cols), 200.0, dtype=BF)
        for gm in l1_groups:
            pos = gm['col'] * 128
            for wi, w in enumerate(gm['wins']):
                k = int(cnt[w])
                sl = slice(off[w], off[w] + k)
                gxs[:, pos:pos + k] = xT[:, s[sl]]
                gxd[:, pos:pos + k] = xT[:, dl[sl] + c * NPC]
                r = np.full(gm['bw'][wi] * 128, 200.0, dtype=np.float32)
                r[:k] = dl[sl] - w * WIN
                rel[:, pos // 128:pos // 128 + gm['bw'][wi]] = \
                    np.ascontiguousarray(
                        r.reshape(gm['bw'][wi], 128).T).astype(BF)
                pos += gm['bw'][wi] * 128
        slabs[c]['gxs'] = gxs
        slabs[c]['gxd'] = gxd
        slabs[c]['rel1'] = rel

    # ---- layer 2: lo/hi sections for int16 gather idxs --------------------
    l2_groups = []
    sorted_cores = []
    for c in range(NCORES):
        s, dl, cnt, off = cores[c]
        sec = (s >= SPLIT).astype(np.int32)
        w = dl >> 7
        order = np.lexsort((sec, w))
        ks, dls = s[order], dl[order]
        cnt2 = np.bincount(w[order] * 2 + sec[order], minlength=NWIN * 2)
        off2 = np.concatenate([[0], np.cumsum(cnt2)])
        sorted_cores.append((ks, dls, cnt2, off2))

    icol = 0
    rcol = 0
    for wins in GROUPS:
        base = wins[0] * WIN
        Blo = [max(-(-int(sorted_cores[c][2][w * 2]) // 128)
                   for c in range(NCORES)) for w in wins]
        Bhi = [max(-(-int(sorted_cores[c][2][w * 2 + 1]) // 128)
                   for c in range(NCORES)) for w in wins]
        nlo, nhi = 128 * sum(Blo), 128 * sum(Bhi)
        ntot = nlo + nhi
        wcols = []
        lo_off, hi_off = 0, nlo // 128
        for wi, w in enumerate(wins):
            cols = (list(range(lo_off, lo_off + Blo[wi]))
                    + list(range(hi_off, hi_off + Bhi[wi])))
            wcols.append(tuple(cols))
            lo_off += Blo[wi]
            hi_off += Bhi[wi]
        l2_groups.append(dict(base=base, nlo=nlo, nhi=nhi, ntot=ntot,
                              wins=tuple(wins), wcols=tuple(wcols),
                              blo=tuple(Blo), bhi=tuple(Bhi),
                              icol=icol, rcol=rcol))
        icol += ntot // 16
        rcol += ntot // 128
    l2_icols, l2_rcols = icol, rcol

    for c in range(NCORES):
        ks, dls, cnt2, off2 = sorted_cores[c]
        idx_parts = []
        rel_parts = []
        for gm in l2_groups:
            ntot = gm['ntot']
            gi = np.zeros(ntot, dtype=np.int32)
            rel = np.full(ntot, 200.0, dtype=np.float32)
            pos = 0
            for pass_hi in (0, 1):
                for wi, w in enumerate(gm['wins']):
                    k = int(cnt2[w * 2 + pass_hi])
                    sl = slice(off2[w * 2 + pass_hi],
                               off2[w * 2 + pass_hi] + k)
                    gi[pos:pos + k] = ks[sl] - (SPLIT if pass_hi else 0)
                    rel[pos:pos + k] = dls[sl] - w * WIN
                    pos += (gm['bhi'] if pass_hi else gm['blo'])[wi] * 128
            idx_parts.append(_wrap_idx(gi))
            rel_parts.append(np.ascontiguousarray(
                rel.reshape(ntot // 128, 128).T).astype(BF))
        slabs[c]['idx2'] = np.ascontiguousarray(
            np.concatenate(idx_parts, axis=1))
        slabs[c]['rel2'] = np.ascontiguousarray(
            np.concatenate(rel_parts, axis=1))

    meta = dict(l1_groups=l1_groups, l1_cols=l1_cols,
                l2_groups=l2_groups, l2_icols=l2_icols, l2_rcols=l2_rcols)
    key = (tuple((g['base'], g['wins'], g['bw'], g['nb'])
                 for g in l1_groups),
           tuple((g['base'], g['nlo'], g['nhi'], g['wins'], g['wcols'])
                 for g in l2_groups))
    return slabs, meta, key


def gat_program(ctx, tc, ins, outs, meta):
    if isinstance(meta, tuple):
        meta = meta[0]
    nc = tc.nc

    t2loc = nc.dram_tensor("t2loc", [NPC, T2C], BF16)
    adT = nc.dram_tensor("adT", [1, NPC], BF16)
    table2 = nc.dram_tensor("table2", [N, T2C], BF16, addr_space="Shared")

    const = ctx.enter_context(tc.tile_pool(name="const", bufs=1))
    gp = ctx.enter_context(tc.tile_pool(name="gp", bufs=3))
    ip = ctx.enter_context(tc.tile_pool(name="ip", bufs=4))
    sm = ctx.enter_context(tc.tile_pool(name="sm", bufs=3))
    ohp = ctx.enter_context(tc.tile_pool(name="ohp", bufs=2))
    wp = ctx.enter_context(tc.tile_pool(name="wp", bufs=2))
    pe = ctx.enter_context(tc.tile_pool(name="pe", bufs=4, space="PSUM"))
    pw = ctx.enter_context(tc.tile_pool(name="pw", bufs=2, space="PSUM"))
    pt = ctx.enter_context(tc.tile_pool(name="pt", bufs=1, space="PSUM"))

    # --- resident constants -------------------------------------------------
    w1t = const.tile([128, F1 + H1], BF16)
    nc.sync.dma_start(out=w1t[:], in_=ins['w1ext'])
    vd1t = const.tile([128, H1], BF16)
    nc.sync.dma_start(out=vd1t[:], in_=ins['vd1'])
    w2t = const.tile([128, F2 + 2 * H2], BF16)
    nc.sync.dma_start(out=w2t[:], in_=ins['w2ext'])
    b1t = const.tile([128, F1], F32)
    nc.sync.dma_start(out=b1t[:], in_=ins['bias1B'])
    b2t = const.tile([128, F2 + 2 * H2], F32)
    nc.sync.dma_start(out=b2t[:], in_=ins['bias2B'])
    onest = const.tile([1, 128], BF16)
    nc.vector.memset(onest[:], 1.0)
    ident = const.tile([128, 128], BF16)
    make_identity(nc, ident[:])
    iota_i = const.tile([128, 128], mybir.dt.int32)
    nc.gpsimd.iota(iota_i[:], pattern=[[1, 128]], base=0, channel_multiplier=0)
    iotaf = const.tile([128, 128], BF16)
    nc.vector.tensor_copy(out=iotaf[:], in_=iota_i[:])

    # --- layer-1 sink: ELU -> transpose -> table2 row chunk ----------------
    def l1_sink(w, nrows, res):
        mn = sm.tile([128, F1], F32, tag="mn")
        nc.vector.tensor_scalar_min(mn[:], res[:], 0.0)
        en = sm.tile([128, F1], F32, tag="en")
        nc.scalar.activation(out=en[:], in_=mn[:],
                             func=mybir.ActivationFunctionType.Exp)
        mp = sm.tile([128, F1], F32, tag="mp")
        nc.vector.tensor_scalar(
            out=mp[:], in0=res[:], scalar1=0.0, scalar2=-1.0,
            op0=mybir.AluOpType.max, op1=mybir.AluOpType.add)
        hp = sm.tile([128, F1], BF16, tag="hp")
        nc.vector.tensor_tensor(
            out=hp[:], in0=en[:], in1=mp[:], op=mybir.AluOpType.add)
        pst = pt.tile([128, 128], BF16, tag="pst")
        nc.tensor.transpose(out=pst[:], in_=hp[:], identity=ident[:])
        hpt = sm.tile([128, 128], BF16, tag="hpt")
        nc.vector.tensor_copy(out=hpt[:], in_=pst[:])
        ps2 = pt.tile([128, F2 + 2 * H2], F32, tag="ps2")
        nc.tensor.matmul(out=ps2[:], lhsT=hpt[:], rhs=w2t[:],
                         start=True, stop=True)
        t2 = sm.tile([128, F2 + 2 * H2], BF16, tag="t2")
        nc.vector.tensor_tensor(
            out=t2[:], in0=ps2[:], in1=b2t[:, 0:F2 + 2 * H2],
            op=mybir.AluOpType.add)
        nc.sync.dma_start(
            out=t2loc.ap()[w * 128:w * 128 + nrows, 0:F2 + 2 * H2],
            in_=t2[:nrows, :])
        # transposed ad2 column for layer-2 adrep broadcasts
        nc.sync.dma_start(
            out=adT.ap()[0:1, w * 128:w * 128 + nrows]
                .rearrange("x n -> n x"),
            in_=t2[:nrows, F2 + H2:F2 + 2 * H2])

    # =======================================================================
    # Layer 1: host-pregathered x slabs -> PE expansion, no gathers
    # =======================================================================
    def l1_consume(gm, oh, wm):
        bcol = 0
        for wi, w in enumerate(gm['wins']):
            nrows = LASTW if w == NWIN - 1 else 128
            bw = gm['bw'][wi]
            pswt = pw.tile([128, 256], F32, tag="psw")
            psw = pswt[:, 0:F1 + H1]
            for j in range(bw):
                nc.tensor.matmul(
                    out=psw, lhsT=oh[:, bcol + j, :], rhs=wm[:, bcol + j, :],
                    start=(j == 0), stop=(j == bw - 1),
                )
            bcol += bw
            rec = sm.tile([128, H1], F32, tag="rec")
            nc.vector.reciprocal(rec[:nrows], psw[:nrows, F1:F1 + H1])
            res = sm.tile([128, F1], F32, tag="res")
            if nrows < 128:
                nc.vector.memset(res[:], 0.0)
            nc.vector.tensor_tensor(
                out=res[:nrows, :].rearrange("p (h c) -> p h c", h=H1),
                in0=psw[:nrows, 0:F1].rearrange("p (h c) -> p h c", h=H1),
                in1=rec[:nrows].unsqueeze(2)
                    .broadcast_to([nrows, H1, F1 // H1]),
                op=mybir.AluOpType.mult,
            )
            nc.vector.tensor_tensor(
                out=res[:nrows, :], in0=res[:nrows, :], in1=b1t[:nrows, :],
                op=mybir.AluOpType.add)
            l1_sink(w, nrows, res)

    pending = None
    for l1gi, gm in enumerate(meta['l1_groups']):
        nb, col = gm['nb'], gm['col']
        gxs = gp.tile([128, nb * 128], BF16, tag="gxs")
        nc.sync.dma_start(out=gxs[:], in_=ins['gxs'][:, col * 128:(col + nb) * 128])
        gxd = gp.tile([128, nb * 128], BF16, tag="gxd")
        nc.sync.dma_start(out=gxd[:], in_=ins['gxd'][:, col * 128:(col + nb) * 128])
        relt = sm.tile([128, nb], BF16, tag="relt")
        nc.sync.dma_start(out=relt[:], in_=ins['rel1'][:, col:col + nb])

        oh = ohp.tile([128, nb, 128], BF16, tag="oh")
        nc.vector.tensor_tensor(
            out=oh[:],
            in0=relt[:].unsqueeze(2).broadcast_to([128, nb, 128]),
            in1=iotaf[:].unsqueeze(1).broadcast_to([128, nb, 128]),
            op=mybir.AluOpType.is_equal,
        )

        wm = wp.tile([128, nb, F1 + H1], BF16, tag="wm")
        # process in PSUM sub-batches of SG blocks
        for s0 in range(0, nb, SG):
            sn = min(SG, nb - s0)
            psA = pe.tile([128, SG, F1 + H1], F32, tag="psA")
            for j in range(sn):
                b = s0 + j
                nc.tensor.matmul(
                    out=psA[:, j, 0:F1 + H1],
                    lhsT=gxs[:, b * 128:(b + 1) * 128],
                    rhs=w1t[:], start=True, stop=False)
                # accumulate alpha_dst onto the alpha_src columns: the
                # PSUM region [F1:F1+H1] ends up holding s = a_src + a_dst
                nc.tensor.matmul(
                    out=psA[:, j, F1:F1 + H1],
                    lhsT=gxd[:, b * 128:(b + 1) * 128],
                    rhs=vd1t[:], start=False, stop=True)
            # leaky_relu(s) = max(0.2*s, s); each op reads PSUM only once
            t1 = sm.tile([128, SG, H1], F32, tag="t1")
            nc.vector.tensor_scalar_mul(t1[:, :sn, :],
                                        psA[:, :sn, F1:F1 + H1], 0.2)
            e = sm.tile([128, SG, H1], F32, tag="e")
            nc.vector.tensor_tensor(
                out=e[:, :sn, :], in0=psA[:, :sn, F1:F1 + H1],
                in1=t1[:, :sn, :], op=mybir.AluOpType.max)
            exb = sm.tile([128, SG, H1], BF16, tag="exb")
            nc.scalar.activation(out=exb[:, :sn, :], in_=e[:, :sn, :],
                                 func=mybir.ActivationFunctionType.Exp)
            nc.vector.tensor_tensor(
                out=wm[:, s0:s0 + sn, 0:F1]
                    .rearrange("p b (h c) -> p b h c", h=H1),
                in0=psA[:, :sn, 0:F1]
                    .rearrange("p b (h c) -> p b h c", h=H1),
                in1=exb[:, :sn, :].unsqueeze(3)
                    .broadcast_to([128, sn, H1, F1 // H1]),
                op=mybir.AluOpType.mult,
            )
            nc.vector.tensor_copy(out=wm[:, s0:s0 + sn, F1:F1 + H1],
                                  in_=exb[:, :sn, :])

        # software pipeline: aggregate+sink the PREVIOUS group so its
        # aggregation matmuls overlap this group's expansion/vector work
        if pending is not None:
            l1_consume(*pending)
        pending = (gm, oh, wm)
    l1_consume(*pending)

    nc.gpsimd.collective_compute(
        "AllGather", mybir.AluOpType.bypass,
        replica_groups=[list(range(NCORES))],
        ins=[t2loc.ap().opt()], outs=[table2.ap().opt()],
    )

    # =======================================================================
    # Layer 2: gather table2 rows by src; alpha_dst via onehot-reduce
    # =======================================================================
    qrr = [0]
    MAXG = 3840
    FD, HD = F2, H2
    for gm in meta['l2_groups']:
        nlo, nhi, ntot, base = gm['nlo'], gm['nhi'], gm['ntot'], gm['base']
        nb = ntot // 128
        icol, rcol = gm['icol'], gm['rcol']
        idxt = ip.tile([128, ntot // 16], I16, tag="idxt")
        nc.sync.dma_start(out=idxt[:], in_=ins['idx2'][:, icol:icol + ntot // 16])
        relt = sm.tile([128, nb], BF16, tag="relt2")
        nc.sync.dma_start(out=relt[:], in_=ins['rel2'][:, rcol:rcol + nb])

        def chunked_gather(out_tile, col0, in_ap, icol0, n, ec):
            for off in range(0, n, MAXG):
                k = min(MAXG, n - off)
                c = col0 + off // 128
                nc.gpsimd.dma_gather(
                    out_ap=out_tile[:, c:c + k // 128, :],
                    in_ap=in_ap,
                    idxs_ap=idxt[:, icol0 + off // 16:
                                 icol0 + (off + k) // 16],
                    num_idxs=k, num_idxs_reg=k,
                    elem_size=ec, elem_step=ec,
                    single_packet=False,
                    queue_num=qrr[0],
                )
                qrr[0] = (qrr[0] + 1) % 4

        g = gp.tile([128, nb, T2C], BF16, tag="g2")
        if nlo:
            chunked_gather(g, 0, table2.ap()[0:SPLIT, :], 0, nlo, T2C)
        if nhi:
            chunked_gather(g, nlo // 128, table2.ap()[SPLIT:N, :],
                           nlo // 16, nhi, T2C)

        # adrep: [128, G*WIN] replicated ad2 row via PE outer product
        adend = min(base + G * WIN, NPC)
        adw = adend - base
        adv = sm.tile([1, 256], BF16, tag="adv")
        nc.sync.dma_start(out=adv[:, :adw], in_=adT.ap()[0:1, base:adend])
        adrep = sm.tile([128, 256], BF16, tag="adrep")
        if adw < 256:
            nc.vector.memset(adrep[:], 0.0)
        nc.gpsimd.partition_broadcast(adrep[:, :adw], adv[:, :adw])

        oh = ohp.tile([128, nb, 128], BF16, tag="oh2")
        nc.vector.tensor_tensor(
            out=oh[:],
            in0=relt[:].unsqueeze(2).broadcast_to([128, nb, 128]),
            in1=iotaf[:].unsqueeze(1).broadcast_to([128, nb, 128]),
            op=mybir.AluOpType.is_equal,
        )
        # alpha_d per edge: reduce_d(oh * adrep_window), batched over the
        # contiguous lo/hi block runs of each window
        ad_e = sm.tile([128, nb, 1], F32, tag="ad_e")
        prod = sm.tile([128, nb, 128], BF16, tag="prod")
        runs = []
        lo_acc, hi_acc = 0, nlo // 128
        for wi, w in enumerate(gm['wins']):
            woff = (w - gm['wins'][0]) * 128
            if gm['blo'][wi]:
                runs.append((lo_acc, gm['blo'][wi], woff))
            if gm['bhi'][wi]:
                runs.append((hi_acc, gm['bhi'][wi], woff))
            lo_acc += gm['blo'][wi]
            hi_acc += gm['bhi'][wi]
        for b0, r, woff in runs:
            nc.vector.tensor_tensor(
                out=prod[:, b0:b0 + r, :], in0=oh[:, b0:b0 + r, :],
                in1=adrep[:, woff:woff + 128].unsqueeze(1)
                    .broadcast_to([128, r, 128]),
                op=mybir.AluOpType.mult)
            nc.vector.tensor_reduce(
                out=ad_e[:, b0:b0 + r, :], in_=prod[:, b0:b0 + r, :],
                axis=mybir.AxisListType.X, op=mybir.AluOpType.add)
        s = sm.tile([128, nb, HD], F32, tag="s2")
        nc.vector.tensor_tensor(
            out=s[:], in0=g[:, :, FD:FD + H2],
            in1=ad_e[:], op=mybir.AluOpType.add)
        e = sm.tile([128, nb, HD], F32, tag="e2")
        nc.vector.scalar_tensor_tensor(
            out=e[:], in0=s[:], scalar=0.2, in1=s[:],
            op0=mybir.AluOpType.mult, op1=mybir.AluOpType.max)
        exb = sm.tile([128, nb, HD], BF16, tag="exb2")
        nc.scalar.activation(out=exb[:], in_=e[:],
                             func=mybir.ActivationFunctionType.Exp)
        wm = wp.tile([128, nb, FD + HD], BF16, tag="wm2")
        nc.vector.tensor_tensor(
            out=wm[:, :, 0:FD],
            in0=g[:, :, 0:FD],
            in1=exb[:].broadcast_to([128, nb, FD]),
            op=mybir.AluOpType.mult,
        )
        nc.vector.tensor_copy(out=wm[:, :, FD:FD + HD], in_=exb[:])

        for wi, w in enumerate(gm['wins']):
            nrows = LASTW if w == NWIN - 1 else 128
            cols = gm['wcols'][wi]
            pswt = pw.tile([128, 256], F32, tag="psw")
            psw = pswt[:, 0:FD + HD]
            for j, b in enumerate(cols):
                nc.tensor.matmul(
                    out=psw, lhsT=oh[:, b, :], rhs=wm[:, b, :],
                    start=(j == 0), stop=(j == len(cols) - 1),
                )
            rec = sm.tile([128, HD], F32, tag="rec2")
            nc.vector.reciprocal(rec[:nrows], psw[:nrows, FD:FD + HD])
            res = sm.tile([128, FD], F32, tag="res2")
            nc.vector.tensor_tensor(
                out=res[:nrows, :],
                in0=psw[:nrows, 0:FD],
                in1=rec[:nrows].broadcast_to([nrows, FD]),
                op=mybir.AluOpType.mult,
            )
            nc.sync.dma_start(
                out=outs['out'][w * 128:w * 128 + nrows, :],
                in_=res[:nrows, 0:FD])


def prepare_host_inputs(x, edge_index, W1, a_src1, a_dst1, b1, W2, a_src2,
                        a_dst2, b2):
    slabs, meta, key = _build_structures(np.asarray(edge_index), x)
    v_s1 = np.einsum('ihc,hc->ih', np.asarray(W1).reshape(F1, H1, F1 // H1),
                     np.asarray(a_src1)).astype(np.float32)
    v_d1 = np.einsum('ihc,hc->ih', np.asarray(W1).reshape(F1, H1, F1 // H1),
                     np.asarray(a_dst1)).astype(np.float32)
    v_s2 = np.einsum('ihc,hc->ih', np.asarray(W2).reshape(F1, H2, F2 // H2),
                     np.asarray(a_src2)).astype(np.float32)
    v_d2 = np.einsum('ihc,hc->ih', np.asarray(W2).reshape(F1, H2, F2 // H2),
                     np.asarray(a_dst2)).astype(np.float32)
    w1ext = np.concatenate(
        [np.asarray(W1, np.float32), v_s1], axis=1).astype(BF)
    w2ext = np.concatenate(
        [np.asarray(W2, np.float32), v_s2, v_d2], axis=1).astype(BF)
    bias1B = np.ascontiguousarray(np.broadcast_to(
        np.asarray(b1, np.float32), (128, F1)))
    bias2 = np.concatenate([np.asarray(b2, np.float32),
                            np.zeros(2 * H2, np.float32)])
    bias2B = np.ascontiguousarray(np.broadcast_to(bias2, (128, F2 + 2 * H2)))
    shared = dict(w1ext=w1ext, vd1=v_d1.astype(BF), w2ext=w2ext,
                  bias1B=bias1B, bias2B=bias2B)
    in_maps = []
    for c in range(NCORES):
        in_maps.append(dict(
            shared,
            gxs=slabs[c]['gxs'], gxd=slabs[c]['gxd'], rel1=slabs[c]['rel1'],
            idx2=slabs[c]['idx2'], rel2=slabs[c]['rel2'],
        ))
    return in_maps, (meta, key)


_PROGRAM_CACHE = {}


def _build_full_program(meta_key):
    meta, key = meta_key
    if key in _PROGRAM_CACHE:
        return _PROGRAM_CACHE[key]
    nc = bacc.Bacc(trn_type="TRN2", num_devices=NCORES, debug=False,
                   num_swdge_queues=4)
    ins = {
        'gxs': nc.dram_tensor('gxs', [128, meta['l1_cols'] * 128], BF16,
                              kind="ExternalInput").ap(),
        'gxd': nc.dram_tensor('gxd', [128, meta['l1_cols'] * 128], BF16,
                              kind="ExternalInput").ap(),
        'rel1': nc.dram_tensor('rel1', [128, meta['l1_cols']], BF16,
                               kind="ExternalInput").ap(),
        'idx2': nc.dram_tensor('idx2', [128, meta['l2_icols']], I16,
                               kind="ExternalInput").ap(),
        'rel2': nc.dram_tensor('rel2', [128, meta['l2_rcols']], BF16,
                               kind="ExternalInput").ap(),
        'w1ext': nc.dram_tensor('w1ext', [128, F1 + H1], BF16,
                                kind="ExternalInput").ap(),
        'vd1': nc.dram_tensor('vd1', [128, H1], BF16,
                              kind="ExternalInput").ap(),
        'w2ext': nc.dram_tensor('w2ext', [128, F2 + 2 * H2], BF16,
                                kind="ExternalInput").ap(),
        'bias1B': nc.dram_tensor('bias1B', [128, F1], F32,
                                 kind="ExternalInput").ap(),
        'bias2B': nc.dram_tensor('bias2B', [128, F2 + 2 * H2], F32,
                                 kind="ExternalInput").ap(),
    }
    outs = {'out': nc.dram_tensor('out', [NPC, F2], F32,
                                  kind="ExternalOutput").ap()}
    with tile.TileContext(nc) as tc:
        with ExitStack() as ctx:
            gat_program(ctx, tc, ins, outs, meta)
    nc.compile()
    _PROGRAM_CACHE[key] = nc
    return nc


def kernel(**inputs) -> np.ndarray:
    in_maps, meta_key = prepare_host_inputs(**inputs)
    nc = _build_full_program(meta_key)
    res = run_bass_kernel_spmd(nc, in_maps, core_ids=list(range(NCORES)))
    return np.concatenate([r['out'] for r in res.results], axis=0)
